# revision 10
# baseline (speedup 1.0000x reference)
"""ParallelRetention fused kernel for 8 Trainium2 NeuronCores.

Strategy (sequence parallel, everything in feature-major "transposed" layout):
  - Shard N=8192 rows across 8 cores (1024 rows each).
  - Per core: compute K^T shard (d x j_r) first and AllGather it (bf16);
    V shard (j_r x v, bf16) + AllGather; Q^T/32 (bf16) and x3^T (fp32r,
    both i-blocks) overlap the collectives.
  - Attention per i-block (512 cols): S^T = K^T.T @ Q^T (bf16 matmuls, fp32
    PSUM), mask-multiply with host-transposed D^T (bf16), exp -> E^T (bf16),
    x1^T = V.T @ E^T accumulated in PSUM; row-sums via ones-vector matmul
    folded into the first E@V pass; normalize with broadcast reciprocal.
  - Tail per i-block: x2^T = gelu(Wr @ x1^T + br2) (fp32r),
    x4^T = Wp_a @ x2^T + Wp_b @ x3^T + bp with fused PReLU (ACT Prelu, per-
    channel alpha) and free per-channel stat accumulation (ACT accum_out).
  - GroupNorm: per-channel sum/sumsq partials AllReduced (8KB), group
    reduction + channel broadcast via tiny one-hot matmuls, then a single
    fused tensor_scalar (x*A + B) pass; x4 rows prefetched under the AR.
  - Host reassembles: out[rows_r] = outT_r.T  (plus D^T/h^T prep).

Biases: bq/32 folded into Wq scale; bv folded into br2 = br + Wr @ bv
(softmax rows sum to 1); bk/bc/bp applied on-chip (per-partition ACT bias).
"""

import ml_dtypes
import numpy as np

import concourse.bacc as bacc
import concourse.tile as tile
from concourse import mybir
from concourse.bass_utils import run_bass_kernel_spmd

dt = mybir.dt
AF = mybir.ActivationFunctionType
ALU = mybir.AluOpType

R = 8            # cores
N = 8192         # rows total
NR = N // R      # rows per core
C = 1024         # all feature dims
P = 128          # partitions
IB = 512         # i-block width
NIB = NR // IB   # 2
JT = N // P      # 64 j-tiles
NC8 = C // P     # 8 chunks of any feature dim
G = 16           # groups
GSZ = C // G     # 64 channels per group
STAT_K = 1.0 / (GSZ * N)  # 1/524288
EPS = 1e-5

_CACHED_NC = None


def _r3(ap_2d):
    """[8*128, W] dram slice -> [p, chunk, W] iteration order for one DMA."""
    return ap_2d.rearrange("(c p) q -> p c q", p=P)


def build_nc():
    nc = bacc.Bacc(None, num_devices=R)

    # ---- DRAM I/O ----
    hT = nc.dram_tensor("hT", [C, NR], dt.bfloat16, kind="ExternalInput")
    hpT = nc.dram_tensor("hpT", [C, NR], dt.float32r, kind="ExternalInput")
    DT = nc.dram_tensor("DT", [N, NR], dt.bfloat16, kind="ExternalInput")
    WqT32 = nc.dram_tensor("WqT32", [C, C], dt.bfloat16, kind="ExternalInput")
    WkT = nc.dram_tensor("WkT", [C, C], dt.bfloat16, kind="ExternalInput")
    WvT = nc.dram_tensor("WvT", [C, C], dt.bfloat16, kind="ExternalInput")
    WrT = nc.dram_tensor("WrT", [C, C], dt.float32r, kind="ExternalInput")
    WcT = nc.dram_tensor("WcT", [C, C], dt.float32r, kind="ExternalInput")
    WpTa = nc.dram_tensor("WpTa", [C, C], dt.float32r, kind="ExternalInput")
    WpTb = nc.dram_tensor("WpTb", [C, C], dt.float32r, kind="ExternalInput")
    bq32 = nc.dram_tensor("bq32", [C, 1], dt.float32, kind="ExternalInput")
    bk_d = nc.dram_tensor("bk", [C, 1], dt.float32, kind="ExternalInput")
    br2_d = nc.dram_tensor("br2", [C, 1], dt.float32, kind="ExternalInput")
    bc_d = nc.dram_tensor("bc", [C, 1], dt.float32, kind="ExternalInput")
    bp_d = nc.dram_tensor("bp", [C, 1], dt.float32, kind="ExternalInput")
    pw_d = nc.dram_tensor("pw", [C, 1], dt.float32, kind="ExternalInput")
    gw_d = nc.dram_tensor("gw", [C, 1], dt.float32, kind="ExternalInput")
    gb_d = nc.dram_tensor("gb", [C, 1], dt.float32, kind="ExternalInput")
    Gm_d = nc.dram_tensor("Gm", [C, G], dt.float32r, kind="ExternalInput")
    GT_d = nc.dram_tensor("GTm", [G, C], dt.float32r, kind="ExternalInput")
    outT = nc.dram_tensor("outT", [C, NR], dt.float32, kind="ExternalOutput")

    # ---- internal DRAM ----
    k_in = nc.dram_tensor("k_in", [C, NR], dt.bfloat16)
    k_ag = nc.dram_tensor("k_ag", [R * C, NR], dt.bfloat16, addr_space="Shared")
    v_in = nc.dram_tensor("v_in", [NR, C], dt.bfloat16)
    v_ag = nc.dram_tensor("v_ag", [R * NR, C], dt.bfloat16, addr_space="Shared")
    qt_d = nc.dram_tensor("qt_d", [C, NR], dt.bfloat16)
    x3_d = nc.dram_tensor("x3_d", [C, NR], dt.float32r)
    x4_d = nc.dram_tensor("x4_d", [C, NR], dt.float32)
    st_in = nc.dram_tensor("st_in", [C, 2], dt.float32r)
    st_out = nc.dram_tensor("st_out", [C, 2], dt.float32r, addr_space="Shared")

    rg = [list(range(R))]

    with tile.TileContext(nc, num_cores=R) as tc:
        with tc.tile_pool(name="persist", bufs=1) as pp:
            ones_col_b = pp.tile([P, 1], dt.bfloat16, name="ones_col_b")
            nc.vector.memset(ones_col_b[:], 1.0)
            ones_row_f = pp.tile([1, P], dt.float32, name="ones_row_f")
            nc.vector.memset(ones_row_f[:], 1.0)
            ones_row = pp.tile([1, P], dt.float32r, name="ones_row")
            nc.vector.tensor_copy(ones_row[:], ones_row_f[:])
            zero2 = pp.tile([P, 2], dt.float32, name="zero2")
            nc.vector.memset(zero2[:], 0.0)
            s_acc = []
            for ct in range(NC8):
                t = pp.tile([P, 2], dt.float32r, name=f"s_acc{ct}")
                nc.vector.tensor_copy(t[:], zero2[:])
                s_acc.append(t)

            # ================= Stage P: projections =================
            with (
                tc.tile_pool(name="sbP", bufs=1) as sbP,
                tc.tile_pool(name="wP", bufs=3) as wP,
                tc.tile_pool(name="evP", bufs=3) as evP,
                tc.tile_pool(name="psP", bufs=4, space="PSUM") as psP,
            ):
                ht = []
                for cc in range(NC8):
                    t = sbP.tile([P, NR], dt.bfloat16, name=f"ht{cc}")
                    nc.sync.dma_start(t[:], hT[cc * P:(cc + 1) * P, :])
                    ht.append(t)
                bkt = sbP.tile([P, NC8, 1], dt.float32, name="bkt")
                nc.sync.dma_start(bkt[:], _r3(bk_d[:, :]))
                wk = []
                for dtile in range(NC8):
                    t = sbP.tile([P, NC8, P], dt.bfloat16, name=f"wk{dtile}")
                    nc.sync.dma_start(t[:], _r3(WkT[:, dtile * P:(dtile + 1) * P]))
                    wk.append(t)
                # preload every other stage-P operand now so the V/Q/x3
                # projections run purely from SBUF while the AllGathers own HBM
                wv = []
                for cc in range(NC8):
                    t = sbP.tile([P, C], dt.bfloat16, name=f"wv{cc}")
                    nc.sync.dma_start(t[:], WvT[cc * P:(cc + 1) * P, :])
                    wv.append(t)
                wq = []
                for dtile in range(NC8):
                    t = sbP.tile([P, NC8, P], dt.bfloat16, name=f"wq{dtile}")
                    nc.sync.dma_start(t[:], _r3(WqT32[:, dtile * P:(dtile + 1) * P]))
                    wq.append(t)
                wc = []
                for htile in range(NC8):
                    t = sbP.tile([P, NC8, P], dt.float32r, name=f"wc{htile}")
                    nc.sync.dma_start(t[:], _r3(WcT[:, htile * P:(htile + 1) * P]))
                    wc.append(t)
                hp = []
                for cc in range(NC8):
                    t = sbP.tile([P, NR], dt.float32r, name=f"hp{cc}")
                    nc.sync.dma_start(t[:], hpT[cc * P:(cc + 1) * P, :])
                    hp.append(t)
                bqt = sbP.tile([P, NC8, 1], dt.float32, name="bqt")
                nc.sync.dma_start(bqt[:], _r3(bq32[:, :]))
                bctl = sbP.tile([P, NC8, 1], dt.float32, name="bctl")
                nc.sync.dma_start(bctl[:], _r3(bc_d[:, :]))
                for jb in range(NR // IB):
                    for dtile in range(NC8):
                        ps = psP.tile([P, IB], dt.float32, name="kps", tag="pps")
                        for cc in range(NC8):
                            nc.tensor.matmul(
                                ps[:], wk[dtile][:, cc, :], ht[cc][:, jb * IB:(jb + 1) * IB],
                                start=(cc == 0), stop=(cc == NC8 - 1))
                        ev = evP.tile([P, IB], dt.bfloat16, name="kev")
                        nc.scalar.activation(ev[:], ps[:], AF.Identity,
                                             bias=bkt[:, dtile, :])
                        nc.sync.dma_start(
                            k_in[dtile * P:(dtile + 1) * P, jb * IB:(jb + 1) * IB], ev[:])
                nc.gpsimd.collective_compute(
                    "AllGather", ALU.bypass, ins=[k_in[:].opt()],
                    outs=[k_ag[:].opt()], replica_groups=rg)

                # V shard: [j_r, v] = h @ Wv.T  (no bias; folded into br2)
                for vb in range(C // IB):
                    for jt8 in range(NC8):
                        ps = psP.tile([P, IB], dt.float32, name="vps", tag="pps")
                        for cc in range(NC8):
                            nc.tensor.matmul(
                                ps[:], ht[cc][:, jt8 * P:(jt8 + 1) * P],
                                wv[cc][:, vb * IB:(vb + 1) * IB],
                                start=(cc == 0), stop=(cc == NC8 - 1))
                        ev = evP.tile([P, IB], dt.bfloat16, name="vev")
                        nc.scalar.copy(ev[:], ps[:])
                        nc.sync.dma_start(
                            v_in[jt8 * P:(jt8 + 1) * P, vb * IB:(vb + 1) * IB], ev[:])
                nc.gpsimd.collective_compute(
                    "AllGather", ALU.bypass, ins=[v_in[:].opt()],
                    outs=[v_ag[:].opt()], replica_groups=rg)

                # Q^T/32: [d, i] = Wq32 @ h^T (+bq/32); overlaps AllGathers
                for dtile in range(NC8):
                    wq_t = wq[dtile]
                    for jb in range(NR // IB):
                        ps = psP.tile([P, IB], dt.float32, name="qps", tag="pps")
                        for cc in range(NC8):
                            nc.tensor.matmul(
                                ps[:], wq_t[:, cc, :], ht[cc][:, jb * IB:(jb + 1) * IB],
                                start=(cc == 0), stop=(cc == NC8 - 1))
                        ev = evP.tile([P, IB], dt.bfloat16, name="qev")
                        nc.scalar.activation(ev[:], ps[:], AF.Identity,
                                             bias=bqt[:, dtile, :])
                        nc.sync.dma_start(
                            qt_d[dtile * P:(dtile + 1) * P, jb * IB:(jb + 1) * IB], ev[:])

                # x3^T (both i-blocks): Wc @ h'^T + bc -> x3_d; overlaps AGs
                for htile in range(NC8):
                    wc_t = wc[htile]
                    for jb in range(NR // IB):
                        ps = psP.tile([P, IB], dt.float32, name="x3ps", tag="pps")
                        for cc in range(NC8):
                            nc.tensor.matmul(
                                ps[:], wc_t[:, cc, :], hp[cc][:, jb * IB:(jb + 1) * IB],
                                start=(cc == 0), stop=(cc == NC8 - 1))
                        ev = evP.tile([P, IB], dt.float32r, name="x3ev")
                        nc.scalar.activation(ev[:], ps[:], AF.Identity,
                                             bias=bctl[:, htile, :])
                        nc.sync.dma_start(
                            x3_d[htile * P:(htile + 1) * P, jb * IB:(jb + 1) * IB], ev[:])

            # ================= Stage A: attention + tail =================
            with (
                tc.tile_pool(name="sbA", bufs=1) as sbA,
                tc.tile_pool(name="etA", bufs=1) as etA,
                tc.tile_pool(name="stream", bufs=3) as stream,
                tc.tile_pool(name="wT", bufs=2) as wT,
                tc.tile_pool(name="psA", bufs=2, space="PSUM") as psA,
            ):
                br2t = sbA.tile([P, NC8, 1], dt.float32, name="br2t")
                nc.sync.dma_start(br2t[:], _r3(br2_d[:, :]))
                bpt = sbA.tile([P, NC8, 1], dt.float32, name="bpt")
                nc.sync.dma_start(bpt[:], _r3(bp_d[:, :]))
                pwt = sbA.tile([P, NC8, 1], dt.float32, name="pwt")
                nc.sync.dma_start(pwt[:], _r3(pw_d[:, :]))

                for ib in range(NIB):
                    i0 = ib * IB
                    qti = sbA.tile([P, NC8, IB], dt.bfloat16, name="qti")
                    nc.sync.dma_start(qti[:], _r3(qt_d[:, i0:i0 + IB]))

                    # ---- A1: scores -> mask -> exp ----
                    ets = {}
                    for jt in range(JT):
                        rank, jl = jt // NC8, jt % NC8
                        kt = stream.tile([P, NC8, P], dt.bfloat16, name="kt", bufs=4)
                        nc.sync.dma_start(
                            kt[:],
                            _r3(k_ag[rank * C:(rank + 1) * C, jl * P:(jl + 1) * P]))
                        dtt = stream.tile([P, IB], dt.bfloat16, name="dtt", bufs=4)
                        nc.sync.dma_start(dtt[:], DT[jt * P:(jt + 1) * P, i0:i0 + IB])
                        sps = psA.tile([P, IB], dt.float32, name="sps", tag="sps", bufs=3)
                        for dc in range(NC8):
                            nc.tensor.matmul(
                                sps[:], kt[:, dc, :], qti[:, dc, :],
                                start=(dc == 0), stop=(dc == NC8 - 1))
                        mt = stream.tile([P, IB], dt.float32, name="mt", bufs=3)
                        nc.vector.tensor_mul(mt[:], sps[:], dtt[:])
                        et = etA.tile([P, IB], dt.bfloat16, name=f"et{jt}")
                        nc.scalar.activation(et[:], mt[:], AF.Exp)
                        ets[jt] = et

                    # ---- A2: x1^T = V.T @ E^T (+ row-sums in pass 0) ----
                    rsps = psA.tile([1, IB], dt.float32, name="rsps", tag="rsps", bufs=1)
                    x1 = []
                    recipbc = sbA.tile([P, IB], dt.float32, name="recipbc")
                    for vh in range(2):
                        vaccs = [psA.tile([P, IB], dt.float32, name=f"vacc{k}",
                                          tag="vacc", bufs=4) for k in range(4)]
                        for jt in range(JT):
                            vt = stream.tile([P, IB], dt.bfloat16, name="vt", bufs=6)
                            nc.scalar.dma_start(
                                vt[:], v_ag[jt * P:(jt + 1) * P,
                                            vh * IB:(vh + 1) * IB])
                            for k in range(4):
                                nc.tensor.matmul(
                                    vaccs[k][:], vt[:, k * P:(k + 1) * P], ets[jt][:],
                                    start=(jt == 0), stop=(jt == JT - 1))
                            if vh == 0:
                                nc.tensor.matmul(
                                    rsps[:], ones_col_b[:], ets[jt][:],
                                    start=(jt == 0), stop=(jt == JT - 1))
                        if vh == 0:
                            rec = sbA.tile([1, IB], dt.float32r, name="rec")
                            with nc.allow_low_precision(reason="f32r matches f32 width"):
                                nc.vector.reciprocal(rec[:], rsps[:])
                            bcp = psA.tile([P, IB], dt.float32, name="bcp", tag="sps", bufs=3)
                            nc.tensor.matmul(bcp[:], ones_row[:], rec[:], start=True, stop=True)
                            nc.scalar.copy(recipbc[:], bcp[:])
                        for k in range(4):
                            x1t = sbA.tile([P, IB], dt.float32r, name=f"x1_{vh * 4 + k}")
                            nc.vector.tensor_mul(x1t[:], vaccs[k][:], recipbc[:])
                            x1.append(x1t)

                    # ---- tail ----
                    # x2^T = gelu(Wr @ x1^T + br2)
                    x2 = []
                    for htile in range(NC8):
                        wr_t = wT.tile([P, NC8, P], dt.float32r, name="wr_t",
                                       tag="wtail", bufs=3)
                        nc.sync.dma_start(wr_t[:], _r3(WrT[:, htile * P:(htile + 1) * P]))
                        ps = psA.tile([P, IB], dt.float32, name="x2ps", tag="sps", bufs=3)
                        for vc in range(NC8):
                            nc.tensor.matmul(ps[:], wr_t[:, vc, :], x1[vc][:],
                                             start=(vc == 0), stop=(vc == NC8 - 1))
                        t = sbA.tile([P, IB], dt.float32r, name=f"x2_{htile}")
                        nc.scalar.activation(t[:], ps[:], AF.Gelu, bias=br2t[:, htile, :])
                        x2.append(t)
                    # x3 reload for this i-block
                    x3i = sbA.tile([P, NC8, IB], dt.float32r, name="x3i")
                    nc.sync.dma_start(x3i[:], _r3(x3_d[:, i0:i0 + IB]))
                    # x4^T = Wp_a @ x2^T + Wp_b @ x3^T + bp, PReLU, stats
                    for ct in range(NC8):
                        wpa_t = wT.tile([P, NC8, P], dt.float32r, name="wpa_t",
                                        tag="wtail", bufs=3)
                        nc.sync.dma_start(wpa_t[:], _r3(WpTa[:, ct * P:(ct + 1) * P]))
                        wpb_t = wT.tile([P, NC8, P], dt.float32r, name="wpb_t",
                                        tag="wtail", bufs=3)
                        nc.sync.dma_start(wpb_t[:], _r3(WpTb[:, ct * P:(ct + 1) * P]))
                        ps = psA.tile([P, IB], dt.float32, name="x4ps", tag="sps", bufs=3)
                        for hc in range(NC8):
                            nc.tensor.matmul(ps[:], wpa_t[:, hc, :], x2[hc][:],
                                             start=(hc == 0), stop=False)
                        for hc in range(NC8):
                            nc.tensor.matmul(ps[:], wpb_t[:, hc, :], x3i[:, hc, :],
                                             start=False, stop=(hc == NC8 - 1))
                        x4t = sbA.tile([P, IB], dt.float32, name="x4t", bufs=2)
                        s1p = sbA.tile([P, 1], dt.float32r, name="s1p", bufs=2)
                        with nc.allow_low_precision(reason="f32r accum is f32-width"):
                            nc.scalar.activation(x4t[:], ps[:], AF.Prelu,
                                                 bias=bpt[:, ct, :], alpha=pwt[:, ct, :],
                                                 accum_out=s1p[:])
                        sqt = sbA.tile([P, IB], dt.float32, name="sqt", bufs=2)
                        s2p = sbA.tile([P, 1], dt.float32r, name="s2p", bufs=2)
                        with nc.allow_low_precision(reason="f32r accum is f32-width"):
                            nc.scalar.activation(sqt[:], x4t[:], AF.Square,
                                                 accum_out=s2p[:])
                        nc.vector.tensor_add(s_acc[ct][:, 0:1], s_acc[ct][:, 0:1], s1p[:])
                        nc.vector.tensor_add(s_acc[ct][:, 1:2], s_acc[ct][:, 1:2], s2p[:])
                        nc.sync.dma_start(x4_d[ct * P:(ct + 1) * P, i0:i0 + IB], x4t[:])

            # ================= Stage G: GroupNorm =================
            with (
                tc.tile_pool(name="sbG", bufs=1) as sbG,
                tc.tile_pool(name="strG", bufs=2) as strG,
                tc.tile_pool(name="psG", bufs=2, space="PSUM") as psG,
            ):
                for ct in range(NC8):
                    nc.sync.dma_start(st_in[ct * P:(ct + 1) * P, :], s_acc[ct][:])
                nc.gpsimd.collective_compute(
                    "AllReduce", ALU.add, ins=[st_in[:].opt()],
                    outs=[st_out[:].opt()], replica_groups=rg)

                # prefetch x4 rows + per-channel affine inputs under the AR
                x4rows = []
                for ct in range(NC8):
                    t = sbG.tile([P, NR], dt.float32, name=f"x4row{ct}")
                    nc.scalar.dma_start(t[:], x4_d[ct * P:(ct + 1) * P, :])
                    x4rows.append(t)
                gwt = sbG.tile([P, NC8, 1], dt.float32, name="gwt")
                nc.sync.dma_start(gwt[:], _r3(gw_d[:, :]))
                gbt = sbG.tile([P, NC8, 1], dt.float32, name="gbt")
                nc.sync.dma_start(gbt[:], _r3(gb_d[:, :]))

                gps = psG.tile([G, 2], dt.float32, name="gps", tag="gps")
                for cc in range(NC8):
                    stt = strG.tile([P, 2], dt.float32r, name="stt")
                    nc.sync.dma_start(stt[:], st_out[cc * P:(cc + 1) * P, :])
                    gmt = strG.tile([P, G], dt.float32r, name="gmt")
                    nc.sync.dma_start(gmt[:], Gm_d[cc * P:(cc + 1) * P, :])
                    nc.tensor.matmul(gps[:], gmt[:], stt[:],
                                     start=(cc == 0), stop=(cc == NC8 - 1))
                mstats = sbG.tile([G, 2], dt.float32, name="mstats")
                nc.scalar.mul(mstats[:], gps[:], STAT_K)   # [mean, E[x^2]]
                m2 = sbG.tile([G, 1], dt.float32, name="m2")
                nc.scalar.activation(m2[:], mstats[:, 0:1], AF.Square)
                vart = sbG.tile([G, 1], dt.float32, name="vart")
                nc.vector.tensor_sub(vart[:], mstats[:, 1:2], m2[:])
                vare = sbG.tile([G, 1], dt.float32, name="vare")
                nc.vector.tensor_scalar_add(vare[:], vart[:], EPS)
                # rsqrt via exp(-0.5 * ln(x)) — high-precision table set
                lnv = sbG.tile([G, 1], dt.float32, name="lnv")
                nc.scalar.activation(lnv[:], vare[:], AF.Ln)
                inv = sbG.tile([G, 1], dt.float32, name="inv")
                nc.scalar.activation(inv[:], lnv[:], AF.Exp, scale=-0.5)
                mi = sbG.tile([G, 2], dt.float32r, name="mi")
                nc.vector.tensor_copy(mi[:, 0:1], mstats[:, 0:1])
                nc.vector.tensor_copy(mi[:, 1:2], inv[:])

                for ct in range(NC8):
                    gtt = strG.tile([G, P], dt.float32r, name="gtt")
                    nc.sync.dma_start(gtt[:], GT_d[:, ct * P:(ct + 1) * P])
                    bps = psG.tile([P, 2], dt.float32, name="bps", tag="gps")
                    nc.tensor.matmul(bps[:], gtt[:], mi[:], start=True, stop=True)
                    mc = sbG.tile([P, 2], dt.float32, name="mc", bufs=2)
                    nc.vector.tensor_copy(mc[:], bps[:])
                    a_col = sbG.tile([P, 1], dt.float32, name="a_col", bufs=2)
                    nc.vector.tensor_mul(a_col[:], mc[:, 1:2], gwt[:, ct, :])
                    tmp = sbG.tile([P, 1], dt.float32, name="tmp", bufs=2)
                    nc.vector.tensor_mul(tmp[:], mc[:, 0:1], a_col[:])
                    b_col = sbG.tile([P, 1], dt.float32, name="b_col", bufs=2)
                    nc.vector.tensor_sub(b_col[:], gbt[:, ct, :], tmp[:])
                    y = sbG.tile([P, NR], dt.float32, name="y", bufs=2)
                    nc.vector.tensor_scalar(y[:], x4rows[ct][:], a_col[:], b_col[:],
                                            ALU.mult, ALU.add)
                    nc.sync.dma_start(outT[ct * P:(ct + 1) * P, :], y[:])

    nc.compile()
    return nc


def prep_in_maps(h, D, h_prime, Wq, bq, Wk, bk, Wv, bv, Wr, br, Wc, bc,
                 Wp, bp, prelu_w, gn_w, gn_b):
    f = np.float32
    col = lambda v: np.ascontiguousarray(np.asarray(v, f).reshape(-1, 1))
    WqT32 = np.ascontiguousarray(np.asarray(Wq, f).T / 32.0)
    WkTn = np.ascontiguousarray(np.asarray(Wk, f).T)
    WvTn = np.ascontiguousarray(np.asarray(Wv, f).T)
    WrTn = np.ascontiguousarray(np.asarray(Wr, f).T)
    WcTn = np.ascontiguousarray(np.asarray(Wc, f).T)
    WpT = np.asarray(Wp, f).T
    WpTa = np.ascontiguousarray(WpT[:C])
    WpTb = np.ascontiguousarray(WpT[C:])
    br2 = np.asarray(br, f) + np.asarray(Wr, f) @ np.asarray(bv, f)
    Gm = np.zeros((C, G), f)
    Gm[np.arange(C), np.arange(C) // GSZ] = 1.0
    GTm = np.ascontiguousarray(Gm.T)
    b16 = lambda a: a.astype(ml_dtypes.bfloat16)
    shared = {
        "WqT32": b16(WqT32), "WkT": b16(WkTn), "WvT": b16(WvTn),
        "WrT": WrTn, "WcT": WcTn,
        "WpTa": WpTa, "WpTb": WpTb,
        "bq32": col(np.asarray(bq, f) / 32.0), "bk": col(bk), "br2": col(br2),
        "bc": col(bc), "bp": col(bp), "pw": col(prelu_w),
        "gw": col(gn_w), "gb": col(gn_b), "Gm": Gm, "GTm": GTm,
    }
    h = np.asarray(h, f)
    D = np.asarray(D, f)
    hp = np.asarray(h_prime, f)
    in_maps = []
    for r in range(R):
        rows = slice(r * NR, (r + 1) * NR)
        m = dict(shared)
        m["hT"] = np.ascontiguousarray(h[rows].T).astype(ml_dtypes.bfloat16)
        m["hpT"] = np.ascontiguousarray(hp[rows].T)
        m["DT"] = np.ascontiguousarray(D[rows].T).astype(ml_dtypes.bfloat16)
        in_maps.append(m)
    return in_maps


def get_nc():
    global _CACHED_NC
    if _CACHED_NC is None:
        _CACHED_NC = build_nc()
    return _CACHED_NC


def run(in_maps, **kw):
    return run_bass_kernel_spmd(get_nc(), in_maps, list(range(R)), **kw)


def kernel(**inputs):
    in_maps = prep_in_maps(**inputs)
    res = run(in_maps)
    out = np.empty((N, C), np.float32)
    for r in range(R):
        out[r * NR:(r + 1) * NR, :] = res.results[r]["outT"].T
    return out
